# revision 12
# baseline (speedup 1.0000x reference)
"""Trainium2 Bass kernel for SPADE/SEAN-style normalization block (nn_ACE_41094247088736).

kernel(**inputs) takes FULL unsharded numpy inputs (as produced by
reference.setup_inputs()) and returns the FULL [4,128,128,128] float32 output.

Sharding: 8 cores; core i handles batch b=i//2, row half h=i%2 (64 rows).
All parameters replicated. No collectives: instance-norm statistics are
computed per-core from a redundant load of the other row-half.

Math (per core, batch b, rows r0..r0+63):
  mu[j,e]   = relu(sum_d sc[b,j,d] fc_w[j,e,d] + fc_b[j,e])
  Pg[j,dy,dx,c] = sum_e mu[j,e] Wga[c,e,dy,dx]      (ditto Pb with Wba)
  gamma_avg = conv19(seg, Pg)   [= conv512(middle_avg, Wga), by linearity]
  actv      = relu(conv19(seg, Wsh) + b_sh)         (bias via extra K-channel)
  gamma_sp  = conv512(actv, Wgs)
  out       = xn*(1 + ga*gamma_avg + (1-ga)*gamma_sp + gbias) + (beta...)

v2 speedups over the bf16 baseline:
  * conv512 (the dominant cost) runs in fp8e4 with perf_mode=DoubleRow:
    virtual K=256 (2 k-tiles of 128), halving the matmul count. Weights are
    host-prescaled by 2^7 and actv by 2^4 (exact powers of two); the 2^-11
    undo is folded into the blend scale.
  * the avg path accumulates into the SAME psum as the spade path (pg is
    pre-scaled on device by ga/(1-ga)*2^11), removing the separate avg conv
    pass + 32 staging tiles; psum is consumed by one scale+bias activation.
  * x stays resident in SBUF after the stats pass (no second HBM read).
"""

import math
import os
from contextlib import ExitStack

import numpy as np
import ml_dtypes

import concourse.bass as bass
from concourse import bacc
import concourse.tile as tile
import concourse.mybir as mybir
from concourse.bass_utils import run_bass_kernel_spmd

BF16 = mybir.dt.bfloat16
F32 = mybir.dt.float32
FP8 = mybir.dt.float8e4
AF = mybir.ActivationFunctionType
ALU = mybir.AluOpType
PM = mybir.MatmulPerfMode
NPBF = ml_dtypes.bfloat16
NPF8 = ml_dtypes.float8_e4m3

B, C, H, W = 4, 128, 128, 128
L, S = 19, 512
EPS = 1e-5
NCORES = 8
ROWS = 64          # owned rows per core
AR = 68            # actv rows (owned + 2 halo each side)
SR = 72            # seg rows  (owned + 4 halo each side)
WPAD = 132         # x with 2 pad cols each side
ASCALE = 16.0      # actv fp8 prescale (2^4)
WSCALE = 128.0     # spade conv weight fp8 prescale (2^7)
UNDO = 1.0 / (ASCALE * WSCALE)   # 2^-11


# ----------------------------------------------------------------------------
# Device graph
# ----------------------------------------------------------------------------

def build_graph(reps=1, actv_reps=1, proj_reps=1, main_reps=1, group=4):
    nc = bacc.Bacc("TRN2", target_bir_lowering=False, debug=False,
                   num_devices=NCORES)

    def din(name, shape, dt):
        return nc.dram_tensor(name, shape, dt, kind="ExternalInput").ap()

    x_own = din("x_own", [128, 16, 512], F32)
    x_oth = din("x_oth", [128, 16, 512], F32)
    seg_d = din("seg_dx", [96, SR, WPAD], BF16)
    seg8_d = din("seg8", [96, SR, WPAD], FP8)
    sct_d = din("sct", [128, 4, L], BF16)
    fcb_d = din("fcb", [128, 4, L], F32)
    fcw_d = din("fcw", [L, 128, 4, 4, 128], BF16)       # [j, di, dc, ec, ei]
    wsh_d = din("wsh", [96, 3, 2, 4, 128], FP8)         # [(j,dx)|u, dyp, ko, sc, ci]
    wsp_d = din("wsp", [128, 2, 2, 5, 5, 2, 128], FP8)  # [ki, cv, g, dy, dx, ko, ci]
    wavg_d = din("wavg", [2, 4, 128, 5, 5, 128], BF16)  # [cv, ec, ei, dy, dx, ci]
    bvec_d = din("bvec", [128, 4], F32)                 # [bga, bba, bgs, bbs]
    blend_d = din("blend", [1, 2], F32)
    out_d = nc.dram_tensor("out", [128, 16, 512], F32,
                           kind="ExternalOutput").ap()

    with tile.TileContext(nc) as tc, ExitStack() as ctx:
        main = ctx.enter_context(tc.tile_pool(name="main", bufs=1))

        # ---- small constants -------------------------------------------------
        seg_sb = main.tile([96, SR, WPAD], BF16)
        nc.sync.dma_start(seg_sb[:], seg_d[:])
        seg8_sb = main.tile([96, SR, WPAD], FP8)
        nc.sync.dma_start(seg8_sb[:], seg8_d[:])
        wsh_sb = main.tile([96, 3, 2, 4, 128], FP8)
        nc.sync.dma_start(wsh_sb[:], wsh_d[:])
        wsp_sb = main.tile([128, 2, 2, 5, 5, 2, 128], FP8)
        nc.scalar.dma_start(wsp_sb[:], wsp_d[:])
        sct_sb = main.tile([128, 4, L], BF16)
        nc.sync.dma_start(sct_sb[:], sct_d[:])
        fcb_sb = main.tile([128, 4, L], F32)
        nc.sync.dma_start(fcb_sb[:], fcb_d[:])
        bvec_sb = main.tile([128, 4], F32)
        nc.sync.dma_start(bvec_sb[:], bvec_d[:])
        bl_sb = main.tile([128, 2], F32)
        nc.sync.dma_start(
            bl_sb[:],
            bass.AP(tensor=blend_d.tensor, offset=blend_d.offset,
                    ap=[[0, 128], blend_d.ap[1]]))

        eps_sb = main.tile([128, 1], F32)
        nc.vector.memset(eps_sb[:], EPS)

        # sigmoid blending factors, broadcast on all partitions
        gab = main.tile([128, 2], F32)
        nc.scalar.activation(gab[:], bl_sb[:], AF.Sigmoid)
        omg = main.tile([128, 2], F32)   # [1-ga, 1-ba]
        nc.vector.tensor_scalar(omg[:], in0=gab[:], scalar1=-1.0, scalar2=1.0,
                                op0=ALU.mult, op1=ALU.add)
        # blend constants: cgam = ga*bga + (1-ga)*bgs + 1 ; cbet = ba*bba + (1-ba)*bbs
        cmix = main.tile([128, 2], F32)
        t_av = main.tile([128, 2], F32)
        nc.vector.tensor_mul(t_av[:], bvec_sb[:, 0:2], gab[:])
        nc.vector.tensor_mul(cmix[:], bvec_sb[:, 2:4], omg[:])
        nc.vector.tensor_add(cmix[:], cmix[:], t_av[:])
        nc.vector.tensor_scalar(cmix[:, 0:1], in0=cmix[:, 0:1], scalar1=1.0,
                                scalar2=0.0, op0=ALU.add, op1=ALU.add)
        cgam = cmix[:, 0:1]
        cbet = cmix[:, 1:2]
        # psum consumption scale: omgs = (1-mix) * 2^-11
        omgs = main.tile([128, 2], F32)
        nc.vector.tensor_scalar(omgs[:], in0=omg[:], scalar1=UNDO, scalar2=0.0,
                                op0=ALU.mult, op1=ALU.add)
        # pg prescale: mixfac = mix/(1-mix) * 2^11  (per cv)
        mixfac = main.tile([128, 2], F32)
        nc.vector.reciprocal(mixfac[:], omg[:])
        nc.vector.tensor_mul(mixfac[:], mixfac[:], gab[:])
        nc.vector.tensor_scalar(mixfac[:], in0=mixfac[:], scalar1=1.0 / UNDO,
                                scalar2=0.0, op0=ALU.mult, op1=ALU.add)

        x_sb = main.tile([128, 16, 512], F32)

        for rep in range(reps):
            # prefetch this rep's x (consumed by phase 4 stats + phase 5 blend)
            nc.sync.dma_start(x_sb[:], x_own[:])

            # ---- phase 2: actv = relu(conv19(seg, wsh) + bias) * 16, fp8 ---------
            actv_sb = []
            for g in range(2):
                a = main.tile([128, 2, AR, WPAD], FP8, tag=f"actv{g}",
                              name=f"actv{g}")
                nc.vector.memset(a[:], 0.0)
                actv_sb.append(a)
            # conv19 as fp8 DoubleRow: k-tile pairs are (dy, dy+1) row-shifted
            # views of seg (one-hot -> exact in fp8); the odd 5th tap pairs
            # with a stride-0 dup whose weights are zero. Weights prescaled
            # by 2^7; relu undoes it (scale = 2^4 / 2^7).
            def seg_pair_rhs(t0, p):
                s0 = seg8_sb[:, t0 + 2 * p:t0 + 2 * p + 1, 2:130]
                return bass.AP(
                    tensor=s0.tensor, offset=s0.offset,
                    ap=[s0.ap[0], [WPAD if p < 2 else 0, 2], [WPAD, 4],
                        [1, 128]])

            for ar in range(actv_reps):
                GA = 4
                with tc.tile_pool(name=f"apsum{rep}_{ar}", bufs=GA + 1,
                                  space="PSUM") as apool:
                    for sc in range(4):
                        g, ko = sc // 2, sc % 2
                        for it0 in range(0, AR // 4, GA):
                            gn = min(GA, AR // 4 - it0)
                            pss = [apool.tile([128, 512], F32, tag="aps",
                                              name="aps") for _ in range(gn)]
                            for p in range(3):
                                for gi in range(gn):
                                    t0 = (it0 + gi) * 4
                                    nc.tensor.matmul(
                                        pss[gi][:], wsh_sb[:, p, :, sc, :],
                                        seg_pair_rhs(t0, p),
                                        start=(p == 0), stop=(p == 2),
                                        perf_mode=PM.DoubleRow)
                            for gi in range(gn):
                                t0 = (it0 + gi) * 4
                                nc.scalar.activation(
                                    actv_sb[g][:, ko, t0:t0 + 4, 2:130],
                                    pss[gi][:].rearrange("p (r c) -> p r c", r=4),
                                    AF.Relu, scale=ASCALE / 128.0)

            # ---- phase 1: mu (style MLP), output muT [e, j] bf16 -----------------
            muT_sb = main.tile([128, 4, L], BF16)
            for pr1 in range(proj_reps):
                with tc.tile_pool(name=f"mups{rep}_{pr1}", bufs=1, space="PSUM") as mupool, \
                     tc.tile_pool(name=f"fcpool{rep}_{pr1}", bufs=4) as fcpool:
                    mups = []
                    for ec in range(4):
                        mups.append(mupool.tile([128, L], F32, tag=f"mu{ec}", name=f"mu{ec}"))
                    # one accumulation group per psum bank: start only on the very
                    # first matmul into each mups[ec] tile, stop on the last
                    # (PSUM 'start' zeroes the whole 2KB zero-region).
                    for j0 in range(0, L, 2):
                        jn = min(2, L - j0)
                        fct = fcpool.tile([128, 2, 4, 4, 128], BF16, tag="fct")
                        nc.gpsimd.dma_start(fct[:, 0:jn], fcw_d[j0:j0 + jn].rearrange("a b c d e -> b a c d e"))
                        for ji in range(jn):
                            j = j0 + ji
                            for ec in range(4):
                                for dc in range(4):
                                    nc.tensor.matmul(mups[ec][:, j:j + 1],
                                                     fct[:, ji, dc, ec, :],
                                                     sct_sb[:, dc, j:j + 1],
                                                     start=(j == 0 and dc == 0),
                                                     stop=(j == L - 1 and dc == 3))
                    tmp = main.tile([128, 4, L], F32)
                    for ec in range(4):
                        nc.vector.tensor_add(tmp[:, ec, :], mups[ec][:], fcb_sb[:, ec, :])
                    nc.scalar.activation(muT_sb[:], tmp[:], AF.Relu)

            # ---- phase 3: Pg / Pb projections, prescaled by mixfac ---------------
            pg_l = []
            for cv in range(2):
                p = main.tile([96, 5, 128], BF16, tag=f"pg{cv}", name=f"pg{cv}")
                nc.vector.memset(p[:], 0.0)
                pg_l.append(p)
            for pr3 in range(proj_reps):
                with tc.tile_pool(name=f"pgps{rep}_{pr3}", bufs=1, space="PSUM") as pgpool, \
                     tc.tile_pool(name=f"wavgp{rep}_{pr3}", bufs=3) as wpool, \
                     tc.tile_pool(name=f"pgstage{rep}_{pr3}", bufs=2) as stpool:
                    for cv in range(2):
                        for (h0, hn) in ((0, 3), (3, 2)):
                            ps = pgpool.tile([L, hn, 5, 128], F32, tag=f"pgp{h0}", name=f"pgp{h0}")
                            # 128-f32 slots s=dyi*5+dx pack 4 per 2KB psum bank;
                            # one accumulation group per bank: start on the first
                            # matmul touching the bank, stop on the last.
                            smax = hn * 5 - 1
                            for ec in range(4):
                                wt = wpool.tile([128, 5, 5, 128], BF16, tag="wavg")
                                nc.scalar.dma_start(wt[:], wavg_d[cv, ec])
                                for dyi in range(hn):
                                    for dx in range(5):
                                        s = dyi * 5 + dx
                                        nc.tensor.matmul(
                                            ps[:, dyi, dx, :], muT_sb[:, ec, :],
                                            wt[:, h0 + dyi, dx, :],
                                            start=(ec == 0 and s % 4 == 0),
                                            stop=(ec == 3 and (s % 4 == 3 or s == smax)))
                            st = stpool.tile([L, 3, 5, 128], BF16, tag="pgst", name="pgst")
                            nc.scalar.activation(st[:, 0:hn], ps[:, 0:hn],
                                                 AF.Identity,
                                                 scale=mixfac[0:L, cv:cv + 1])
                            for dyi in range(hn):
                                nc.sync.dma_start(
                                    pg_l[cv][0:95, h0 + dyi, :],
                                    st[:, dyi, :, :])

            # ---- phase 4: instance norm stats ------------------------------------
            stats = main.tile([128, 32, 6], F32)
            for k in range(16):
                nc.vector.bn_stats(stats[:, k, :], x_sb[:, k, :])
            with tc.tile_pool(name=f"xoth{rep}", bufs=2) as xop:
                for c4 in range(4):
                    xt = xop.tile([128, 4, 512], F32, tag="xo")
                    nc.scalar.dma_start(xt[:], x_oth[:, c4 * 4:(c4 + 1) * 4, :])
                    for k in range(4):
                        nc.vector.bn_stats(stats[:, 16 + c4 * 4 + k, :], xt[:, k, :])
            mv = main.tile([128, 2], F32)
            nc.vector.bn_aggr(mv[:], stats[:])
            sd = main.tile([128, 1], F32)
            nc.scalar.activation(sd[:], mv[:, 1:2], AF.Sqrt, bias=eps_sb[:], scale=1.0)
            rstd = main.tile([128, 1], F32)
            nc.vector.reciprocal(rstd[:], sd[:])
            nmr = main.tile([128, 1], F32)   # -mean*rstd
            nc.vector.tensor_scalar(nmr[:], in0=mv[:, 0:1], scalar1=rstd[:],
                                    scalar2=-1.0, op0=ALU.mult, op1=ALU.mult)

            # ---- phase 5: fused spade+avg conv / blend loop ----------------------
            # per (cv, chunk): one psum accumulates 50 fp8 DoubleRow matmuls
            # (conv512 over actv, virtual K=256) + 5 bf16 matmuls (conv19 over
            # seg with mixfac-prescaled pg). Consumption: one scale+bias
            # activation. Spade weights stay stationary across G chunks.
            G = group
            NCH = ROWS // 4
            for mr in range(main_reps):
                with tc.tile_pool(name=f"cpsum{rep}_{mr}", bufs=2 * G, space="PSUM") as cpool, \
                     tc.tile_pool(name=f"blp{rep}_{mr}", bufs=3) as blp:
                    for n0 in range(0, NCH, G):
                        g = min(G, NCH - n0)
                        sps = {(cv, ci): cpool.tile([128, 512], F32, tag="cps",
                                                    name="cps")
                               for cv in range(2) for ci in range(g)}
                        for cv in range(2):
                            i = 0
                            for gg in range(2):
                                for dy in range(5):
                                    for dx in range(5):
                                        for ci in range(g):
                                            yo0 = (n0 + ci) * 4
                                            nc.tensor.matmul(
                                                sps[(cv, ci)][:],
                                                wsp_sb[:, cv, gg, dy, dx],
                                                actv_sb[gg][:, :, yo0 + dy:yo0 + dy + 4,
                                                            dx:dx + 128],
                                                start=(i == 0), stop=False,
                                                perf_mode=PM.DoubleRow)
                                        i += 1
                            for dy in range(5):
                                for ci in range(g):
                                    yo0 = (n0 + ci) * 4
                                    nc.tensor.matmul(
                                        sps[(cv, ci)][:], pg_l[cv][:, dy, :],
                                        seg_sb[:, yo0 + dy + 2:yo0 + dy + 6, 2:130],
                                        start=False, stop=(dy == 4))
                        for ci in range(g):
                            n = n0 + ci
                            gf1 = blp.tile([128, 512], F32, tag="gf1")
                            nc.scalar.activation(gf1[:], sps[(0, ci)][:],
                                                 AF.Identity, bias=cgam,
                                                 scale=omgs[:, 0:1])
                            bf = blp.tile([128, 512], F32, tag="bf")
                            nc.scalar.activation(bf[:], sps[(1, ci)][:],
                                                 AF.Identity, bias=cbet,
                                                 scale=omgs[:, 1:2])
                            xn = blp.tile([128, 512], F32, tag="xn")
                            nc.scalar.activation(xn[:], x_sb[:, n, :],
                                                 AF.Identity, bias=nmr,
                                                 scale=rstd)
                            oo = blp.tile([128, 512], F32, tag="oo")
                            nc.vector.tensor_mul(oo[:], xn[:], gf1[:])
                            nc.vector.tensor_add(oo[:], oo[:], bf[:])
                            nc.sync.dma_start(out_d[:, n, :], oo[:])

    nc.compile()
    return nc


# ----------------------------------------------------------------------------
# Host-side sharding / layout prep
# ----------------------------------------------------------------------------

def _pack_wsp(w):
    """[C=128, S=512, 5, 5] OIHW -> [ki, g, dy, dx, ko, ci], s = g*256+ko*128+ki."""
    a = w.reshape(128, 2, 2, 128, 5, 5)          # [ci, g, ko, ki, dy, dx]
    return a.transpose(3, 1, 4, 5, 2, 0)         # [ki, g, dy, dx, ko, ci]


def _prep_shared(fc_w, fc_b, conv_gamma_w, conv_beta_w,
                 spade_shared_w, spade_shared_b,
                 spade_gamma_w, spade_beta_w,
                 conv_gamma_b, conv_beta_b, spade_gamma_b, spade_beta_b,
                 blending_gamma, blending_beta):
    """Replicated (core-independent) input tensors."""
    fcw = np.ascontiguousarray(
        fc_w.reshape(L, 4, 128, 4, 128).transpose(0, 4, 3, 1, 2)
    ).astype(NPBF)                                          # [j, di, dc, ec, ei]
    fcb = np.ascontiguousarray(
        fc_b.reshape(L, 4, 128).transpose(2, 1, 0)).astype(np.float32)

    wsh5 = np.zeros((96, 5, 4, 128), np.float32)
    # spade_shared_w: [S, L, 5, 5] (OIHW)
    w = spade_shared_w.reshape(4, 128, L, 5, 5)             # [sc, ci, j, dy, dx]
    for dx in range(5):
        # partition p=j*5+dx ; wsh5[p, dy, sc, ci] = w[sc, ci, j, dy, dx]
        wsh5[dx:95:5] = w[:, :, :, :, dx].transpose(2, 3, 0, 1)
    wsh5[95, 2] = spade_shared_b.reshape(4, 128)            # bias row, dy==2 only
    # dy-pair packed fp8 (x128): [q, dyp, ko, sc, ci]; tap = 2*dyp+ko, 6th = 0
    wsh = np.zeros((96, 3, 2, 4, 128), np.float32)
    for p in range(3):
        for ko in range(2):
            if 2 * p + ko < 5:
                wsh[:, p, ko] = wsh5[:, 2 * p + ko]
    wsh = np.clip(wsh * 128.0, -240.0, 240.0).astype(NPF8)

    wsp = np.stack([_pack_wsp(spade_gamma_w),
                    _pack_wsp(spade_beta_w)], axis=1)       # [ki, cv, g, dy, dx, ko, ci]
    wsp = np.clip(wsp * WSCALE, -240.0, 240.0).astype(NPF8)

    wavg = np.stack([
        conv_gamma_w.reshape(128, 4, 128, 5, 5).transpose(1, 2, 3, 4, 0),
        conv_beta_w.reshape(128, 4, 128, 5, 5).transpose(1, 2, 3, 4, 0),
    ], axis=0).astype(NPBF)                                 # [cv, ec, ei, dy, dx, ci]

    bvec = np.stack([conv_gamma_b, conv_beta_b,
                     spade_gamma_b, spade_beta_b], axis=1).astype(np.float32)
    blend = np.array([[float(blending_gamma[0]),
                       float(blending_beta[0])]], np.float32)
    return dict(fcw=np.ascontiguousarray(fcw), fcb=fcb,
                wsh=np.ascontiguousarray(wsh),
                wsp=np.ascontiguousarray(wsp),
                wavg=np.ascontiguousarray(wavg), bvec=bvec, blend=blend)


def _prep_core(core, x, segmap, style_codes):
    b, h = core // 2, core % 2
    r0 = h * ROWS
    x_own = np.ascontiguousarray(
        x[b, :, r0:r0 + ROWS, :].reshape(128, 16, 512)).astype(np.float32)
    o0 = (1 - h) * ROWS
    x_oth = np.ascontiguousarray(
        x[b, :, o0:o0 + ROWS, :].reshape(128, 16, 512)).astype(np.float32)

    # seg_dx: [96, SR, WPAD]; partition j*5+dx; row s <-> global row r0-4+s
    seg_dx = np.zeros((96, SR, WPAD), np.float32)
    g0 = r0 - 4
    lo = max(0, g0)
    hi = min(H, g0 + SR)
    sm = segmap[b][:, lo:hi, :]                             # [L, rows, W]
    padded = np.zeros((L, SR, WPAD), np.float32)
    padded[:, lo - g0:hi - g0, 2:130] = sm
    for dx in range(5):
        sh = np.zeros((L, SR, WPAD), np.float32)
        if dx - 2 < 0:
            sh[:, :, -(dx - 2):] = padded[:, :, :dx - 2]
        elif dx - 2 == 0:
            sh = padded
        else:
            sh[:, :, :-(dx - 2)] = padded[:, :, dx - 2:]
        seg_dx[dx:95:5] = sh
    seg_dx[95, lo - g0:hi - g0, :] = 1.0                    # in-image row flag
    seg8 = seg_dx.astype(NPF8)                              # 0/1 -> exact in fp8
    seg_dx = seg_dx.astype(NPBF)

    sct = np.ascontiguousarray(
        style_codes[b].T.reshape(4, 128, L).transpose(1, 0, 2)).astype(NPBF)
    return dict(x_own=x_own, x_oth=x_oth, seg_dx=np.ascontiguousarray(seg_dx),
                seg8=np.ascontiguousarray(seg8), sct=sct)


_CACHE = {}


def _get_nc():
    if "nc" not in _CACHE:
        _CACHE["nc"] = build_graph()
    return _CACHE["nc"]


def make_in_maps(inputs):
    inputs = {k: np.asarray(v, np.float32) for k, v in inputs.items()}
    shared = _prep_shared(
        inputs["fc_w"], inputs["fc_b"],
        inputs["conv_gamma_w"], inputs["conv_beta_w"],
        inputs["spade_shared_w"], inputs["spade_shared_b"],
        inputs["spade_gamma_w"], inputs["spade_beta_w"],
        inputs["conv_gamma_b"], inputs["conv_beta_b"],
        inputs["spade_gamma_b"], inputs["spade_beta_b"],
        inputs["blending_gamma"], inputs["blending_beta"])
    x = np.asarray(inputs["x"], np.float32)
    segmap = np.asarray(inputs["segmap"], np.float32)
    style = np.asarray(inputs["style_codes"], np.float32)
    in_maps = []
    for core in range(NCORES):
        m = dict(shared)
        m.update(_prep_core(core, x, segmap, style))
        in_maps.append(m)
    return in_maps


def assemble(results):
    out = np.zeros((B, C, H, W), np.float32)
    for core in range(NCORES):
        b, h = core // 2, core % 2
        r0 = h * ROWS
        out[b, :, r0:r0 + ROWS, :] = results[core]["out"].reshape(C, ROWS, W)
    return out


def kernel(**inputs):
    nc = _get_nc()
    in_maps = make_in_maps(inputs)
    res = run_bass_kernel_spmd(nc, in_maps, list(range(NCORES)))
    return assemble(res.results)
